# revision 25
# baseline (speedup 1.0000x reference)
"""Trainium2 Bass kernel for nn_AttentionHyperNet (sparse_attention).

Full-input contract: kernel(**inputs) takes the FULL unsharded inputs and
returns the FULL output [2048, 16, 32] f32. Internally shards the batch dim
across 8 NeuronCores (pure data parallel), replicating the small weights.

Per-core dataflow (BS_L=256 samples, groups of 4 samples = 256 entity tokens):
  - entities PE-transposed to feature-major eT [ED, tokens]
  - fc1 / K / Q projections keep activations feature-major (fp32r GEMMs)
  - V produced token-major (one sample per M-tile; keys on partitions 0..63)
  - logits via 32x32-packed fp32 matmuls; sample s on partition strip 32s
    (16 real q rows + 16 zero rows from zero-padded qT columns)
  - softmax over free dim: exp (ACT, scale folded) -> mask-mul -> reduce ->
    reciprocal -> broadcast-mul; mask tiles are persistent, zeroed once
  - w PE-transposed per head -> wT [keys, (s, qslot)]
  - attnT = per-(s,h) packed matmuls -> feature-major attnT [(h,d), q]
  - every 4 groups: Wout GEMM + fc2 GEMM (fp32r), PE-transpose back to
    token-major, post-mask by agent availability, DMA out
Masking: pre-mask only needs the per-key entity mask (masked-agent rows are
fully zeroed at the end by the post-mask, which subsumes the reference's
all-masked/NaN handling and the attn_out zeroing).
"""

import math
import numpy as np
from contextlib import ExitStack

import concourse.bass as bass
import concourse.mybir as mybir
import concourse.tile as tile
from concourse import bacc
from concourse.masks import make_identity

F32 = mybir.dt.float32
F32R = mybir.dt.float32r
I32 = mybir.dt.int32
AF = mybir.ActivationFunctionType

BS, NE, NA, ED, H, NH, M = 2048, 64, 16, 128, 256, 8, 32
HD = H // NH  # 32
N_CORES = 8
BS_L = BS // N_CORES  # 256
SCALE = 1.0 / math.sqrt(HD)
EPS = 1e-30


def build_nc(bs_l=BS_L, use_f32r=True, repeat=1):
    assert bs_l % 16 == 0
    nc = bacc.Bacc("TRN2", target_bir_lowering=False)

    ent_d = nc.dram_tensor("entities", [bs_l, NE, ED], F32, kind="ExternalInput").ap()
    em_d = nc.dram_tensor("entity_mask", [bs_l, NE], I32, kind="ExternalInput").ap()
    w1_d = nc.dram_tensor("W1", [H, ED], F32, kind="ExternalInput").ap()
    b1_d = nc.dram_tensor("b1", [H], F32, kind="ExternalInput").ap()
    win_d = nc.dram_tensor("Win", [3 * H, H], F32, kind="ExternalInput").ap()
    wout_d = nc.dram_tensor("Wout", [H, H], F32, kind="ExternalInput").ap()
    bout_d = nc.dram_tensor("bout", [H], F32, kind="ExternalInput").ap()
    w2_d = nc.dram_tensor("W2", [M, H], F32, kind="ExternalInput").ap()
    b2_d = nc.dram_tensor("b2", [M], F32, kind="ExternalInput").ap()
    out_d = nc.dram_tensor("out", [bs_l, NA, M], F32, kind="ExternalOutput").ap()
    keepa_d = nc.dram_tensor("keepa_scratch", [bs_l * NA], F32).ap()
    keepe_d = nc.dram_tensor("keepe_scratch", [bs_l, NE], F32).ap()
    keepe8_d = nc.dram_tensor("keepe8_scratch", [bs_l, NH * NE], F32).ap()

    GD = F32R if use_f32r else F32  # dtype of big-GEMM operand tiles

    with tile.TileContext(nc) as tc, ExitStack() as ctx:
        # ---------------- pools ----------------
        wpool = ctx.enter_context(tc.tile_pool(name="weights", bufs=1))
        pre = ctx.enter_context(tc.tile_pool(name="pre", bufs=2))
        ent_p = ctx.enter_context(tc.tile_pool(name="ent", bufs=4))
        eT_p = ctx.enter_context(tc.tile_pool(name="eT", bufs=3))
        x1_p = ctx.enter_context(tc.tile_pool(name="x1", bufs=4))
        kT_p = ctx.enter_context(tc.tile_pool(name="kT", bufs=4))
        qT_p = ctx.enter_context(tc.tile_pool(name="qT", bufs=4))
        v_p = ctx.enter_context(tc.tile_pool(name="v", bufs=3))
        wT_p = ctx.enter_context(tc.tile_pool(name="wT", bufs=3))
        p_p = ctx.enter_context(tc.tile_pool(name="p", bufs=3))
        pm_p = ctx.enter_context(tc.tile_pool(name="pm", bufs=3))
        w_p = ctx.enter_context(tc.tile_pool(name="w", bufs=3))
        sum_p = ctx.enter_context(tc.tile_pool(name="sums", bufs=4))
        
        at_p = ctx.enter_context(tc.tile_pool(name="atacc", bufs=4))
        tail_p = ctx.enter_context(tc.tile_pool(name="tail", bufs=3))
        outm_p = ctx.enter_context(tc.tile_pool(name="outm", bufs=4))
        ka_p = ctx.enter_context(tc.tile_pool(name="ka", bufs=4))

        psA = ctx.enter_context(tc.tile_pool(name="psA", bufs=2, space="PSUM"))
        psBig = ctx.enter_context(tc.tile_pool(name="psBig", bufs=1, space="PSUM"))
        psL = ctx.enter_context(tc.tile_pool(name="psL", bufs=1, space="PSUM"))

        # ---------------- preamble ----------------
        ident = wpool.tile([128, 128], F32, tag="ident")
        make_identity(nc, ident[:])

        def load_transposed(src_ap, rows, cols, tag):
            """src [rows, cols] DRAM -> list over col-blocks of SBUF [128, rows]."""
            tiles = []
            for cb in range(cols // 128):
                t = wpool.tile([128, rows], GD, tag=f"{tag}{cb}", name=f"{tag}{cb}")
                tiles.append(t)
                for rb in range((rows + 127) // 128):
                    rsz = min(128, rows - rb * 128)
                    raw = pre.tile([128, 128], F32, tag="wload", name=f"wl{tag}{cb}{rb}")
                    nc.sync.dma_start(
                        out=raw[:rsz, :],
                        in_=src_ap[rb * 128 : rb * 128 + rsz, cb * 128 : (cb + 1) * 128],
                    )
                    ps = psA.tile([128, 256], F32, tag="psA", name=f"pw{tag}{cb}{rb}")
                    nc.tensor.transpose(ps[:, :rsz], raw[:rsz, :], ident[:rsz, :rsz])
                    nc.scalar.activation(
                        t[:, rb * 128 : rb * 128 + rsz], ps[:, :rsz], AF.Copy
                    )
            return tiles

        w1T = load_transposed(w1_d, H, ED, "w1T")[0]
        wqT = load_transposed(win_d[0:H], H, H, "wqT")
        wkT = load_transposed(win_d[H : 2 * H], H, H, "wkT")
        wvT = load_transposed(win_d[2 * H : 3 * H], H, H, "wvT")
        woT = load_transposed(wout_d, H, H, "woT")
        w2T = load_transposed(w2_d, M, H, "w2T")

        def load_bias(src_ap, n, tag):
            tiles = []
            for bb in range((n + 127) // 128):
                sz = min(128, n - bb * 128)
                t = wpool.tile([128, 1], F32, tag=f"{tag}{bb}", name=f"{tag}{bb}")
                tiles.append(t)
                nc.sync.dma_start(
                    out=t[:sz, :],
                    in_=src_ap[bb * 128 : bb * 128 + sz].rearrange("(p o) -> p o", o=1),
                )
            return tiles

        b1_s = load_bias(b1_d, H, "b1")
        bo_s = load_bias(bout_d, H, "bo")
        b2_s = load_bias(b2_d, M, "b2")

        # keep masks -> DRAM scratch
        n_mt = max(1, bs_l // 128)
        spt = min(128, bs_l)
        for mt in range(n_mt):
            emi = pre.tile([128, NE], I32, tag="emi", name=f"emi{mt}")
            nc.sync.dma_start(out=emi[:spt, :], in_=em_d[mt * 128 : mt * 128 + spt, :])
            kf = pre.tile([128, NE], F32, tag="kf", name=f"kf{mt}")
            nc.vector.tensor_scalar(
                out=kf[:spt, :], in0=emi[:spt, :], scalar1=-1.0, scalar2=1.0,
                op0=mybir.AluOpType.mult, op1=mybir.AluOpType.add,
            )
            nc.sync.dma_start(
                out=keepe_d[mt * 128 : mt * 128 + spt, :], in_=kf[:spt, :]
            )
            kf8 = pre.tile([128, NH * NE], F32, tag="kf8", name=f"kf8{mt}")
            nc.vector.tensor_copy(
                kf8[:spt, :].rearrange("p (h k) -> p h k", h=NH),
                kf[:spt, :].unsqueeze(1).broadcast_to([spt, NH, NE]),
            )
            nc.sync.dma_start(
                out=keepe8_d[mt * 128 : mt * 128 + spt, :], in_=kf8[:spt, :]
            )
            nc.sync.dma_start(
                out=keepa_d[mt * 128 * NA : mt * 128 * NA + spt * NA].rearrange(
                    "(p q) -> p q", q=NA
                ),
                in_=kf[:spt, :NA],
            )

        # persistent mask tiles [128(4s x 32 qslots), 512(8h x 64k)], zeroed once
        mk_t = []
        for i in range(2):
            t = wpool.tile([128, NH * NE], F32, tag=f"mk{i}", name=f"mk{i}")
            nc.gpsimd.memset(t[:], 0.0)
            mk_t.append(t)

        # ---------------- main loop ----------------
        n_groups = bs_l // 4
        ATACC = 4 * 4 * NA  # 256 cols: 4 groups x (4s x 16q)

        rep_ctx = tc.For_i(0, repeat, 1) if repeat > 1 else None
        if rep_ctx is not None:
            rep_ctx.__enter__()

        state = {}   # g -> dict(w_sb, v_sb)
        at_acc = [None]

        ent_tiles = {}

        def emit_ent_dma(g):
            tiles = []
            for tb in range(2):
                ent = ent_p.tile([128, ED], F32, tag="ent", name=f"ent{g}_{tb}")
                nc.sync.dma_start(
                    out=ent[:],
                    in_=ent_d[g * 4 + tb * 2 : g * 4 + tb * 2 + 2].rearrange(
                        "s n e -> (s n) e"
                    ),
                )
                tiles.append(ent)
            ent_tiles[g] = tiles

        def emit_front(g):
            # entities -> eT (feature-major); tiles were DMA'd a group ahead
            eT = eT_p.tile([128, 256], GD, tag="eT", name=f"eT{g}")
            for tb in range(2):
                ent = ent_tiles[g][tb]
                ps = psA.tile([128, 256], F32, tag="psA", name=f"pse{g}_{tb}")
                nc.tensor.transpose(ps[:, :128], ent[:], ident[:])
                nc.scalar.activation(
                    eT[:, tb * 128 : (tb + 1) * 128], ps[:, :128], AF.Copy
                )
            del ent_tiles[g]

            # mask strips into persistent tile
            mkb = mk_t[g % 2]
            for s in range(4):
                nc.sync.dma_start(
                    out=mkb[s * 32 : s * 32 + NA, :],
                    in_=keepe8_d[g * 4 + s]
                    .unsqueeze(0)
                    .broadcast_to([NA, NH * NE]),
                )

            # fc1
            x1T = []
            for hb in range(2):
                ps = psA.tile([128, 256], F32, tag="psA", name=f"psf{g}_{hb}")
                nc.tensor.matmul(
                    ps[:], w1T[:, hb * 128 : (hb + 1) * 128], eT[:],
                    start=True, stop=True,
                )
                x1 = x1_p.tile([128, 256], GD, tag="x1", name=f"x1_{g}_{hb}")
                nc.scalar.activation(x1[:], ps[:], AF.Relu, bias=b1_s[hb][:])
                x1T.append(x1)

            # K projection
            kT = []
            for ob in range(2):
                ps = psA.tile([128, 256], F32, tag="psA", name=f"psk{g}_{ob}")
                for kb in range(2):
                    nc.tensor.matmul(
                        ps[:], wkT[kb][:, ob * 128 : (ob + 1) * 128], x1T[kb][:],
                        start=(kb == 0), stop=(kb == 1),
                    )
                kt = kT_p.tile([128, 256], F32, tag="kT", name=f"kT{g}_{ob}")
                nc.vector.tensor_copy(kt[:], ps[:])
                kT.append(kt)

            # Q projection, agents only, zero-padded to 32 cols/sample
            x1_ag = [
                x1T[kb][:].rearrange("p (s t) -> p s t", s=4)[:, :, :NA]
                for kb in range(2)
            ]
            qT = []
            for ob in range(2):
                ps = psA.tile([128, 256], F32, tag="psA", name=f"psq{g}_{ob}")
                for kb in range(2):
                    nc.tensor.matmul(
                        ps[:, :64], wqT[kb][:, ob * 128 : (ob + 1) * 128], x1_ag[kb],
                        start=(kb == 0), stop=(kb == 1),
                    )
                qt = qT_p.tile([128, 128], F32, tag="qT", name=f"qT{g}_{ob}")
                nc.gpsimd.memset(qt[:], 0.0)
                nc.scalar.activation(
                    qt[:].rearrange("p (s t) -> p s t", s=4)[:, :, :NA],
                    ps[:, :64].rearrange("p (s t) -> p s t", s=4),
                    AF.Copy,
                )
                qT.append(qt)

            # V token-major
            psv = psBig.tile([64, 1024], F32, tag="big", name=f"psv{g}")
            for s in range(4):
                for kb in range(2):
                    nc.tensor.matmul(
                        psv[:, s * 256 : (s + 1) * 256],
                        x1T[kb][:, s * 64 : (s + 1) * 64],
                        wvT[kb][:],
                        start=(kb == 0), stop=(kb == 1),
                    )
            v_sb = v_p.tile([64, 1024], F32, tag="v", name=f"v{g}")
            nc.vector.tensor_copy(v_sb[:, :512], psv[:, :512])
            nc.scalar.activation(v_sb[:, 512:], psv[:, 512:], AF.Copy)

            # logits
            psl = psL.tile([128, 2048], F32, tag="psl", name=f"psl{g}")
            for hb in range(2):
                for h4 in range(4):
                    for s in range(4):
                        nc.tensor.matmul(
                            psl[
                                s * 32 : s * 32 + 32,
                                h4 * 512 + hb * 64 : h4 * 512 + hb * 64 + 64,
                            ],
                            qT[hb][h4 * 32 : (h4 + 1) * 32, s * 32 : (s + 1) * 32],
                            kT[hb][h4 * 32 : (h4 + 1) * 32, s * 64 : (s + 1) * 64],
                            start=True, stop=True,
                            tile_position=(32 * h4, 32 * s),
                        )

            # softmax chain (ACT + DVE)
            p_sb = p_p.tile([128, NH * NE], F32, tag="p", name=f"p{g}")
            nc.scalar.activation(
                p_sb[:].rearrange("p (a b k) -> p a b k", a=4, b=2),
                psl[:].rearrange("p (a c) -> p a c", a=4)[:, :, 0:128].rearrange(
                    "p a (b k) -> p a b k", b=2
                ),
                AF.Exp, scale=SCALE,
            )
            pm = pm_p.tile([128, NH * NE], F32, tag="pm", name=f"pm{g}")
            nc.vector.tensor_mul(pm[:], p_sb[:], mkb[:])
            sums = sum_p.tile([128, NH], F32, tag="sums", name=f"su{g}")
            nc.vector.reduce_sum(
                sums[:], pm[:].rearrange("p (h k) -> p h k", h=NH),
                axis=mybir.AxisListType.X,
            )
            rec = sum_p.tile([128, NH], F32, tag="rec", name=f"re{g}")
            nc.vector.tensor_scalar_add(sums[:], sums[:], EPS)
            nc.vector.reciprocal(rec[:], sums[:])
            w_sb = w_p.tile([128, NH * NE], F32, tag="w", name=f"w{g}")
            nc.vector.tensor_mul(
                w_sb[:].rearrange("p (h k) -> p h k", h=NH),
                pm[:].rearrange("p (h k) -> p h k", h=NH),
                rec[:].unsqueeze(2).broadcast_to([128, NH, NE]),
            )
            state[g] = dict(w_sb=w_sb, v_sb=v_sb)

        def emit_back(g):
            g4 = g % 4
            if g4 == 0:
                at_acc[0] = [
                    at_p.tile([128, ATACC], GD, tag=f"atacc{i}", name=f"atacc{i}_{g}")
                    for i in (0, 1)
                ]
            st = state.pop(g)
            w_sb, v_sb = st["w_sb"], st["v_sb"]

            # transpose w per head-position
            pswt = psBig.tile([64, 1024], F32, tag="big", name=f"pswt{g}")
            for p in range(NH):
                nc.tensor.transpose(
                    pswt[:, p * 128 : (p + 1) * 128],
                    w_sb[:, p * NE : (p + 1) * NE],
                    ident[:],
                )
            wT = wT_p.tile([64, 1024], F32, tag="wT", name=f"wTs{g}")
            nc.scalar.activation(wT[:, :512], pswt[:, :512], AF.Copy)
            nc.vector.tensor_copy(wT[:, 512:], pswt[:, 512:])

            # attnT
            psat = psL.tile([128, 2048], F32, tag="psl", name=f"psat{g}")
            for h in range(NH):
                pos = 2 * (h % 4) + h // 4
                for s in range(4):
                    nc.tensor.matmul(
                        psat[
                            32 * (h % 4) : 32 * (h % 4) + 32,
                            s * 512 + (h // 4) * 16 : s * 512 + (h // 4) * 16 + 16,
                        ],
                        v_sb[:, s * 256 + h * 32 : s * 256 + (h + 1) * 32],
                        wT[:, pos * 128 + s * 32 : pos * 128 + s * 32 + NA],
                        start=True, stop=True,
                        tile_position=(0, 32 * (h % 4)),
                    )
            for i in range(2):
                nc.scalar.activation(
                    at_acc[0][i][:, g4 * 64 : (g4 + 1) * 64],
                    psat[:].rearrange("p (s c) -> p s c", s=4)[
                        :, :, i * 16 : i * 16 + 16
                    ],
                    AF.Copy,
                )

        def emit_tail(gg):
            g = gg * 4 + 3
            aoT = []
            for ob in range(2):
                ps = psA.tile([128, 256], F32, tag="psA", name=f"pao{g}_{ob}")
                for kb in range(2):
                    nc.tensor.matmul(
                        ps[:], woT[kb][:, ob * 128 : (ob + 1) * 128],
                        at_acc[0][kb][:], start=(kb == 0), stop=(kb == 1),
                    )
                ao = tail_p.tile([128, 256], GD, tag="aoT", name=f"ao{g}_{ob}")
                nc.scalar.activation(ao[:], ps[:], AF.Identity, bias=bo_s[ob][:])
                aoT.append(ao)
            pso = psA.tile([128, 256], F32, tag="psA", name=f"pso{g}")
            for kb in range(2):
                nc.tensor.matmul(
                    pso[:M, :], w2T[kb][:], aoT[kb][:],
                    start=(kb == 0), stop=(kb == 1),
                )
            oT = tail_p.tile([M, 256], F32, tag="oT", name=f"oT{g}")
            nc.scalar.activation(
                oT[:], pso[:M, :], AF.Identity, bias=b2_s[0][:M, :]
            )
            for tb in range(2):
                pst = psL.tile([128, 2048], F32, tag="psl", name=f"pst{g}_{tb}")
                nc.tensor.transpose(
                    pst[:, :M], oT[:, tb * 128 : (tb + 1) * 128], ident[:M, :M]
                )
                ka = ka_p.tile([128, 1], F32, tag="ka", name=f"ka{g}_{tb}")
                nc.sync.dma_start(
                    out=ka[:],
                    in_=keepa_d[gg * 256 + tb * 128 : gg * 256 + (tb + 1) * 128]
                    .rearrange("(p o) -> p o", o=1),
                )
                om = outm_p.tile([128, M], F32, tag="outm", name=f"om{g}_{tb}")
                nc.vector.tensor_scalar_mul(om[:], pst[:, :M], ka[:])
                nc.sync.dma_start(
                    out=out_d.rearrange("b q m -> (b q) m")[
                        gg * 256 + tb * 128 : gg * 256 + (tb + 1) * 128
                    ],
                    in_=om[:],
                )

        for g in range(n_groups + 1):
            if g == 0:
                emit_ent_dma(0)
            if g + 1 < n_groups:
                emit_ent_dma(g + 1)
            if g < n_groups:
                emit_front(g)
            if g >= 1:
                emit_back(g - 1)
                if (g - 1) % 4 == 3:
                    emit_tail((g - 1) // 4)

        if rep_ctx is not None:
            rep_ctx.__exit__(None, None, None)

    nc.compile()
    return nc


_NC_CACHE = {}


def get_nc(bs_l=BS_L, use_f32r=True):
    key = (bs_l, use_f32r)
    if key not in _NC_CACHE:
        _NC_CACHE[key] = build_nc(bs_l, use_f32r)
    return _NC_CACHE[key]


def kernel(entities, entity_mask, W1, b1, Win, Wout, bout, W2, b2):
    from concourse.bass_utils import run_bass_kernel_spmd

    entities = np.ascontiguousarray(np.asarray(entities), dtype=np.float32)
    entity_mask = np.ascontiguousarray(np.asarray(entity_mask), dtype=np.int32)
    weights = dict(
        W1=np.asarray(W1, np.float32), b1=np.asarray(b1, np.float32),
        Win=np.asarray(Win, np.float32), Wout=np.asarray(Wout, np.float32),
        bout=np.asarray(bout, np.float32), W2=np.asarray(W2, np.float32),
        b2=np.asarray(b2, np.float32),
    )
    nc = get_nc()
    in_maps = []
    for c in range(N_CORES):
        sl = slice(c * BS_L, (c + 1) * BS_L)
        in_maps.append(
            dict(entities=entities[sl], entity_mask=entity_mask[sl], **weights)
        )
    res = run_bass_kernel_spmd(nc, in_maps, core_ids=list(range(N_CORES)))
    outs = [res.results[c]["out"].reshape(BS_L, NA, M) for c in range(N_CORES)]
    return np.concatenate(outs, axis=0)
